# revision 1
# baseline (speedup 1.0000x reference)
"""Trainium2 Bass kernel for a 3-layer GINE drug encoder (message-passing GNN).

Sharding (8-core SPMD, one NEFF):
  - Nodes split into 8 contiguous shards of N/8; each core owns the edges
    whose destination falls in its shard.  After layers 0 and 1 the per-core
    h shard (bf16) is AllGather'd (one collective per layer: a collective
    blocks the issuing Pool queue for its whole transfer, so fewer/larger
    collectives lose the least time).
  - Within a core, nodes are re-permuted (capacity-constrained LPT on
    in-degree) so each 128-node tile receives a near-equal number of incoming
    edges.  Nodes whose pooled graph lives on a DIFFERENT core (pool-boundary
    nodes) are pinned to the first W slots of the shard: after layer 2 only
    those W rows are AllGather'd (~1MB instead of 25.6MB) and pooling reads
    everything else from the core-local layer-2 output table.
  - Edges are laid out in 128-edge columns grouped by (dst-tile, src-block);
    the 4 blocks are <=32k-row slices of the gathered table (int16 gather
    indices).  Pads are "poisoned" edges whose augmented edge-attr row forces
    e = -1e30 so relu(h+e) == 0.  The x table is stored bf16 padded to H
    columns so layer-0 gathers are also 256B/row.
  - Per 8-column group: e = eT_aug.T @ ew_aug on PE into a PSUM bank (first
    matmul per bank carries start=True, which zeroes the bank); the gathered
    h columns are ACCUMULATED into the same PSUM region with identity
    matmuls on PE (so DVE never touches the add; the bank's last matmul
    carries stop=True); ACT does a fused relu + f32->bf16 copy PSUM->SBUF;
    the scatter one-hots are built with per-column tensor_scalar is_equal
    (2x DVE mode).  Scatter-add is a one-hot matmul per column into the agg
    PSUM bank.
  - MLP runs transposed with stationary weights and BN folded into ACT
    per-partition scale/bias; PE-transpose back, add residual, store bf16.
  - Mean-pool: G/8 graphs per core; local pooled nodes are split into two
    row-segments of the layer-2 output table (separate DRAM tensors, so the
    first segment's gathers only depend on the first half of layer-2's
    stores), the boundary segment reads the small AllGather window;
    segment-sums via one-hot matmuls with an appended ones column for
    counts.  The boundary collective is emitted before the remote segment
    so its Pool stall overlaps local-chunk compute.
"""

import heapq
import math
import os
from contextlib import ExitStack

import ml_dtypes
import numpy as np

import concourse.bacc as bacc
import concourse.bass as bass
import concourse.mybir as mybir
import concourse.tile as tile
from concourse.bass_utils import run_bass_kernel_spmd

F32 = mybir.dt.float32
BF16 = mybir.dt.bfloat16
I32 = mybir.dt.int32
I16 = mybir.dt.int16
BF = ml_dtypes.bfloat16

BN_EPS = 1e-5
POISON = -1.0e30
P = 128
SUPER = 4          # node tiles per PSUM supertile (= gather batch)
EGRP = 4           # columns per e-PSUM / elementwise group
NBLK = 4           # gather blocks: A0 A1 B0 B1


class Cfg:
    def __init__(self, N, E, D0, H, G, R, KB, PTL, PTR, W):
        self.N, self.E, self.D0, self.H, self.G, self.R = N, E, D0, H, G, R
        self.NSH = N // R
        self.TIL = math.ceil(self.NSH / P)
        self.NSH_PAD = self.TIL * P
        self.n_super = math.ceil(self.TIL / SUPER)
        self.BLK = math.ceil(N / NBLK)
        self.blocks = [(b * self.BLK, min((b + 1) * self.BLK, N))
                       for b in range(NBLK)]
        assert all(e - s <= 32000 for s, e in self.blocks)
        self.KB = KB                   # columns per (tile, block)
        self.K = NBLK * KB             # columns per tile
        self.C = self.TIL * self.K     # columns per core
        self.GSH = G // R
        self.NG = math.ceil(self.GSH / P)
        self.PTL, self.PTR = PTL, PTR  # pool tiles (local / remote segment)
        self.PT = PTL + PTR            # overwritten when A/B split is set
        self.W = W                     # pinned boundary window per shard


def _balanced_perm(indeg_local, til, last_cap, pinned):
    """Capacity-constrained LPT with `pinned` local node ids forced into the
    first slots (tile-major).  Returns tile-major slot->orig-local map."""
    caps = [P] * til
    caps[-1] = last_cap
    assigned = [[] for _ in range(til)]
    loads = [0] * til
    pinned_set = set(int(p) for p in pinned)
    for i, node in enumerate(pinned):
        t = i // P
        assigned[t].append(int(node))
        loads[t] += int(indeg_local[node])
    order = np.argsort(-indeg_local, kind="stable")
    heap = [(loads[t], t) for t in range(til)]
    heapq.heapify(heap)
    for node in order:
        if int(node) in pinned_set:
            continue
        d = int(indeg_local[node])
        popped = []
        while True:
            load, t = heapq.heappop(heap)
            if len(assigned[t]) < caps[t]:
                assigned[t].append(int(node))
                heapq.heappush(heap, (load + d, t))
                break
            popped.append((load, t))
        for it in popped:
            heapq.heappush(heap, it)
    perm = np.full(til * P, -1, dtype=np.int64)
    for t in range(til):
        for i, node in enumerate(assigned[t]):
            perm[t * P + i] = node
    return perm


def _wrap16(idx_linear, dst, col0):
    """Place linear int16 index stream at wrapped position: element i ->
    [16k + i%16, col0 + i//16] for k in 0..7 (replicated across Q7 cores)."""
    n = len(idx_linear)
    m = n // 16
    w = idx_linear.reshape(m, 16).T.astype(np.int16)   # [16, m]
    for k in range(8):
        dst[16 * k:16 * (k + 1), col0:col0 + m] = w


def preprocess(x, edge_attr, edge_index, batch, n_graphs, R, super_=SUPER):
    N, D0 = x.shape
    E = edge_index.shape[1]
    G = int(n_graphs)
    src = np.asarray(edge_index[0], dtype=np.int64)
    dst = np.asarray(edge_index[1], dtype=np.int64)
    batch = np.asarray(batch, dtype=np.int64)
    x = np.asarray(x, dtype=np.float32)
    edge_attr = np.asarray(edge_attr, dtype=np.float32)

    NSH = N // R
    TIL = math.ceil(NSH / P)
    last_cap = NSH - (TIL - 1) * P
    GSH = G // R
    indeg = np.bincount(dst, minlength=N)

    # ---- pool ranges & boundary (pinned) nodes -------------------------
    gs = np.searchsorted(batch, np.arange(G + 1))
    pool_lo = gs[np.arange(0, G + 1, GSH)]          # [R+1] node boundaries
    pool_core = np.searchsorted(pool_lo, np.arange(N), side="right") - 1
    owner = np.arange(N) // NSH
    pinned_mask = pool_core != owner
    pinned_by_core = [np.nonzero(pinned_mask[r * NSH:(r + 1) * NSH])[0]
                      for r in range(R)]
    maxpin = max(len(p) for p in pinned_by_core)
    W = max(512, math.ceil(maxpin / (SUPER * P)) * SUPER * P)
    assert W <= 2048, f"boundary window {maxpin} too large"

    # ---- permutation (LPT with pinning) --------------------------------
    newid = np.empty(N, dtype=np.int64)
    orig_of_new = np.empty(N, dtype=np.int64)
    for r in range(R):
        lo = r * NSH
        perm = _balanced_perm(indeg[lo:lo + NSH], TIL, last_cap,
                              pinned_by_core[r])
        real = perm[perm >= 0]
        newid[lo + real] = lo + np.arange(NSH)
        orig_of_new[lo:lo + NSH] = lo + real

    BLK = math.ceil(N / NBLK)
    src_new = newid[src]
    src_blk = src_new // BLK
    src_idxblk = src_new - src_blk * BLK
    dst_new = newid[dst]

    # ---- (core, tile, block) edge buckets + global KB ------------------
    buckets = {}
    KB = 1
    for r in range(R):
        lo = r * NSH
        sel = np.nonzero((dst_new >= lo) & (dst_new < lo + NSH))[0]
        t_of = (dst_new[sel] - lo) // P
        key = t_of * NBLK + src_blk[sel]
        order = np.argsort(key, kind="stable")
        sel = sel[order]
        cnt = np.bincount(key[order], minlength=TIL * NBLK)
        pos = 0
        for t in range(TIL):
            for b in range(NBLK):
                n_tb = int(cnt[t * NBLK + b])
                buckets[(r, t, b)] = sel[pos:pos + n_tb]
                pos += n_tb
                KB = max(KB, math.ceil(n_tb / P))

    # ---- pool slots (local then remote segment) ------------------------
    # local pooled nodes split into two segments at a super-aligned row so
    # segment-A pooling only depends on the first half of layer-2's stores
    # (pool gathers then overlap layer-2's tail).
    n_super_ = math.ceil(TIL / super_)
    RSPLIT = min(((n_super_ + 1) // 2) * super_ * P, TIL * P)
    pool_locA, pool_locB, pool_rem = [], [], []
    for r in range(R):
        a, b_ = int(pool_lo[r]), int(pool_lo[r + 1])
        nodes = np.arange(a, b_)
        own = owner[nodes] == r
        ln = nodes[own]
        rn = nodes[~own]
        lrow = newid[ln] - r * NSH
        inA = lrow < RSPLIT
        pool_locA.append((ln[inA], lrow[inA]))                # ptabA rows
        pool_locB.append((ln[~inA], lrow[~inA] - RSPLIT))     # ptabB rows
        rr = newid[rn] // NSH
        ro = newid[rn] % NSH
        assert np.all(ro < W), "remote pooled node not pinned"
        pool_rem.append((rn, rr * W + ro))                    # bnd_full rows
    PTA = max(1, max(math.ceil(len(q[0]) / P) for q in pool_locA))
    PTB = max(1, max(math.ceil(len(q[0]) / P) for q in pool_locB))
    PTL = PTA + PTB
    PTR = max(1, max(math.ceil(len(q[0]) / P) for q in pool_rem))

    cfg = Cfg(N, E, D0, None, G, R, KB, PTL, PTR, W)
    cfg.PTA, cfg.PTB, cfg.RSPLIT = PTA, PTB, RSPLIT
    cfg.PT = PTA + PTB + PTR
    K, C, PT, NG = cfg.K, cfg.C, cfg.PT, cfg.NG

    x_perm = np.zeros((N, 128), dtype=BF)
    x_perm[:, :D0] = x[orig_of_new].astype(BF)

    per_core = []
    for r in range(R):
        lo = r * NSH
        S = C * P
        idx_lin = np.zeros(S, dtype=np.int64)
        dloc = np.zeros(S, dtype=np.float32)
        attrT = np.zeros((18, S), dtype=np.float32)
        attrT[17, :] = 1.0  # poison flag on pad slots

        n_sup = math.ceil(TIL / super_)
        slot0 = 0
        for s in range(n_sup):
            t0 = s * super_
            nt = min(super_, TIL - t0)
            for b in range(NBLK):
                for ti in range(nt):
                    t = t0 + ti
                    e_ids = buckets[(r, t, b)]
                    n_tb = len(e_ids)
                    sl = slice(slot0, slot0 + n_tb)
                    idx_lin[sl] = src_idxblk[e_ids]
                    dloc[sl] = (dst_new[e_ids] - lo - t * P).astype(np.float32)
                    attrT[0:16, sl] = edge_attr[e_ids].T
                    attrT[16, sl] = 1.0
                    attrT[17, sl] = 0.0
                    slot0 += KB * P
        assert slot0 == S

        idx16 = np.zeros((P, S // 16), dtype=np.int16)
        _wrap16(idx_lin, idx16, 0)
        dloc_t = np.ascontiguousarray(dloc.reshape(C, P).T)   # f32
        eta = np.ascontiguousarray(attrT.astype(BF))

        # pool arrays: segments A (early ptab rows), B, then remote
        SP_ = PT * P
        pool_idx_lin = np.zeros(SP_, dtype=np.int64)
        pool_bl = np.full((NG, SP_), -1.0, dtype=np.float32)
        segs = [(0, pool_locA[r]), (PTA * P, pool_locB[r]),
                ((PTA + PTB) * P, pool_rem[r])]
        for seg0, (nodes, rows) in segs:
            pool_idx_lin[seg0:seg0 + len(rows)] = rows
            blv = (batch[nodes] - r * GSH).astype(np.float32)
            for g in range(NG):
                v = blv - g * P
                pool_bl[g, seg0:seg0 + len(nodes)] = np.where(
                    (v >= 0) & (v < P), v, -1.0)
        pool_idx16 = np.zeros((P, SP_ // 16), dtype=np.int16)
        _wrap16(pool_idx_lin, pool_idx16, 0)
        pbs = np.stack([pool_bl[g].reshape(PT, P).T for g in range(NG)],
                       axis=2)
        pool_bl_t = np.ascontiguousarray(pbs.reshape(P, PT * NG))

        xsh = np.zeros((cfg.NSH_PAD, D0), dtype=np.float32)
        xsh[:NSH] = x[orig_of_new[lo:lo + NSH]]
        per_core.append(dict(
            src_idx=idx16, dstloc=dloc_t, eta=eta,
            pool_idx=pool_idx16, pool_bl=pool_bl_t,
            xsh=np.ascontiguousarray(xsh.astype(BF)),
        ))
    return cfg, x_perm, per_core


# --------------------------------------------------------------------------
# weight folding
# --------------------------------------------------------------------------
def fold_weights(H, D0,
                 mlp0_w1, mlp0_b1, mlp0_bn_g, mlp0_bn_b, mlp0_bn_m, mlp0_bn_v,
                 mlp0_w2, mlp0_b2, edge0_w, edge0_b,
                 mlps_w1, mlps_b1, mlps_bn_g, mlps_bn_b, mlps_bn_m, mlps_bn_v,
                 mlps_w2, mlps_b2, edge_w, edge_b, bn_g, bn_b, bn_m, bn_v):
    f32 = lambda a: np.asarray(a, dtype=np.float32)
    w = {}

    def ew_aug(ew, eb, cout):
        # replicated at each 32-partition stack base (matmul requires
        # lhsT and rhs at the same base partition)
        a = np.zeros((82, cout), dtype=np.float32)
        for st in range(3):
            a[32 * st:32 * st + 16] = f32(ew)
            a[32 * st + 16] = f32(eb)
            a[32 * st + 17] = POISON
        return a

    w["ew_aug0"] = ew_aug(edge0_w, edge0_b, D0)
    w["ew_aug1"] = ew_aug(edge_w[0], edge_b[0], H)
    w["ew_aug2"] = ew_aug(edge_w[1], edge_b[1], H)
    w["w1_0"], w["w2_0"] = f32(mlp0_w1), f32(mlp0_w2)
    w["w1_s"], w["w2_s"] = f32(mlps_w1), f32(mlps_w2)

    si0 = f32(mlp0_bn_g) / np.sqrt(f32(mlp0_bn_v) + BN_EPS)
    w["si0"] = si0
    w["ti0"] = (f32(mlp0_b1) - f32(mlp0_bn_m)) * si0 + f32(mlp0_bn_b)
    sis = f32(mlps_bn_g) / np.sqrt(f32(mlps_bn_v) + BN_EPS)
    w["sis"] = sis
    w["tis"] = (f32(mlps_b1) - f32(mlps_bn_m)) * sis + f32(mlps_bn_b)
    b2 = [f32(mlp0_b2), f32(mlps_b2), f32(mlps_b2)]
    for l in range(3):
        so = f32(bn_g[l]) / np.sqrt(f32(bn_v[l]) + BN_EPS)
        w[f"sout{l}"] = so
        w[f"tout{l}"] = (b2[l] - f32(bn_m[l])) * so + f32(bn_b[l])
    return w


# --------------------------------------------------------------------------
# program builder
# --------------------------------------------------------------------------
def build_program(cfg, weights):
    N, D0, H, R = cfg.N, cfg.D0, cfg.H, cfg.R
    TIL, KB, K, C = cfg.TIL, cfg.KB, cfg.K, cfg.C
    PT, PTL, PTR, NG, GSH = cfg.PT, cfg.PTL, cfg.PTR, cfg.NG, cfg.GSH
    NSH, NSH_PAD, W = cfg.NSH, cfg.NSH_PAD, cfg.W
    n_super = cfg.n_super

    nc = bacc.Bacc("TRN2", target_bir_lowering=False, num_devices=R)

    xin = nc.dram_tensor("x_perm", [N, H], BF16, kind="ExternalInput")
    xsh_t = nc.dram_tensor("xsh", [NSH_PAD, D0], BF16, kind="ExternalInput")
    idx_t = nc.dram_tensor("src_idx", [P, C * P // 16], I16,
                           kind="ExternalInput")
    dl_t = nc.dram_tensor("dstloc", [P, C], F32, kind="ExternalInput")
    eta_t = nc.dram_tensor("eta", [18, C * P], BF16, kind="ExternalInput")
    pidx_t = nc.dram_tensor("pool_idx", [P, PT * P // 16], I16,
                            kind="ExternalInput")
    pbl_t = nc.dram_tensor("pool_bl", [P, PT * NG], F32, kind="ExternalInput")
    out_t = nc.dram_tensor("out", [GSH, H], F32, kind="ExternalOutput")

    hsh = [nc.dram_tensor(f"hsh{l}", [NSH_PAD, H], BF16) for l in range(2)]
    shared = "Shared" if R > 1 else "Local"
    hfull = [nc.dram_tensor(f"hfull{l}", [N, H], BF16, addr_space=shared)
             for l in range(2)]
    RSPLIT = cfg.RSPLIT
    ptabA = nc.dram_tensor("ptabA", [RSPLIT, H], BF16)
    ptabB = nc.dram_tensor("ptabB", [max(NSH_PAD - RSPLIT, P), H], BF16)
    bnd_in = nc.dram_tensor("bnd_in", [W, H], BF16)
    bnd_full = nc.dram_tensor("bnd_full", [R * W, H], BF16, addr_space=shared)

    def inl(name, arr):
        return nc.inline_tensor(np.ascontiguousarray(arr), name=name)

    ew_c = [inl(f"ew_aug{l}", weights[f"ew_aug{l}"].astype(BF))
            for l in range(3)]
    w1_c = [inl("w1_0c", weights["w1_0"].astype(BF)),
            inl("w1_sc", weights["w1_s"].astype(BF))]
    w2_c = [inl("w2_0c", weights["w2_0"].astype(BF)),
            inl("w2_sc", weights["w2_s"].astype(BF))]
    si_c = [inl("si0c", weights["si0"].reshape(H, 1)),
            inl("sisc", weights["sis"].reshape(H, 1))]
    ti_c = [inl("ti0c", weights["ti0"].reshape(H, 1)),
            inl("tisc", weights["tis"].reshape(H, 1))]
    so_c = [inl(f"so{l}c", weights[f"sout{l}"].reshape(H, 1)) for l in range(3)]
    to_c = [inl(f"to{l}c", weights[f"tout{l}"].reshape(H, 1)) for l in range(3)]
    idf_c = inl("identf", np.eye(P, dtype=np.float32))
    idb_c = inl("identb", np.eye(P, dtype=np.float32).astype(BF))
    zrow_c = inl("zrow", np.zeros((1, SUPER * P), dtype=np.float32).astype(BF))
    ones_c = inl("onesc", np.ones((P, 1), dtype=np.float32).astype(BF))
    iota_c = inl("iotab",
                 np.ascontiguousarray(
                     np.broadcast_to(
                         np.arange(2 * P, dtype=np.float32)[None, :],
                         (P, 2 * P)).astype(BF)))

    RG = [list(range(R))]

    with tile.TileContext(nc) as tc, ExitStack() as ctx:
        cp = ctx.enter_context(tc.tile_pool(name="consts", bufs=1))

        def const_tile(dram, shape, dtype, tag):
            t = cp.tile(shape, dtype, tag=tag, name=tag)
            nc.sync.dma_start(out=t[:], in_=dram[:])
            return t

        ew_s = [const_tile(ew_c[l], [82, D0 if l == 0 else H], BF16,
                           f"ew{l}")
                for l in range(3)]
        w1_s = [const_tile(w1_c[0], [D0, H], BF16, "w1a"),
                const_tile(w1_c[1], [H, H], BF16, "w1b")]
        w2_s = [const_tile(w2_c[0], [H, H], BF16, "w2a"),
                const_tile(w2_c[1], [H, H], BF16, "w2b")]
        si_s = [const_tile(si_c[i], [H, 1], F32, f"si{i}") for i in range(2)]
        ti_s = [const_tile(ti_c[i], [H, 1], F32, f"ti{i}") for i in range(2)]
        so_s = [const_tile(so_c[l], [H, 1], F32, f"so{l}") for l in range(3)]
        to_s = [const_tile(to_c[l], [H, 1], F32, f"to{l}") for l in range(3)]
        idf_s = const_tile(idf_c, [P, P], F32, "idf")
        idb_s = const_tile(idb_c, [P, P], BF16, "idb")
        zrow_s = const_tile(zrow_c, [1, SUPER * P], BF16, "zrow")
        ones_s = const_tile(ones_c, [P, 1], BF16, "ones")
        iota_s = const_tile(iota_c, [P, 2 * P], BF16, "iota")

        with ExitStack() as lctx:
            sb = lctx.enter_context(tc.tile_pool(name="work", bufs=2))
            sb4 = lctx.enter_context(tc.tile_pool(name="work4", bufs=4))
            sb3 = lctx.enter_context(tc.tile_pool(name="work3", bufs=3))
            ps_e = lctx.enter_context(tc.tile_pool(name="ps_e", bufs=4,
                                                   space="PSUM"))
            ps_a = lctx.enter_context(tc.tile_pool(name="ps_a", bufs=2,
                                                   space="PSUM"))
            ps_m = lctx.enter_context(tc.tile_pool(name="ps_m", bufs=2,
                                                   space="PSUM"))

            for l in range(3):
                Cin = D0 if l == 0 else H
                table = xin if l == 0 else hfull[l - 1]
                wi = 0 if l == 0 else 1

                for s in range(n_super):
                    t0 = s * SUPER
                    nt = min(SUPER, TIL - t0)
                    ncols = nt * NBLK * KB
                    c0 = t0 * K   # first column of batch

                    dlt = sb4.tile([P, SUPER * K], F32, tag="dl")
                    ett = sb3.tile([18, SUPER * K * P], BF16, tag="et")
                    nc.sync.dma_start(out=dlt[:, :ncols],
                                      in_=dl_t[:, c0:c0 + ncols])
                    nc.sync.dma_start(
                        out=ett[:, :ncols * P],
                        in_=eta_t[:, c0 * P:(c0 + ncols) * P])

                    # gathered h (one dma_gather per (block, 8-col chunk));
                    # the x table is bf16 padded to H columns so layer 0
                    # gathers 256B rows too (dma_gather needs 256B-aligned
                    # elems) and the identity-add runs at bf16 speed.
                    hg = sb4.tile([P, SUPER * K * H], BF16, tag="hg",
                                  name="hg")
                    bcols = nt * KB  # columns per block in this batch
                    GC = 8           # dma_gather cap: 8 cols = 1024 indices
                    it = sb4.tile([P, SUPER * K * 8], I16, tag="it",
                                  name="it")
                    nc.sync.dma_start(out=it[:, :ncols * 8],
                                      in_=idx_t[:, c0 * 8:(c0 + ncols) * 8])
                    for b in range(NBLK):
                        bs, be = cfg.blocks[b]
                        for cc0 in range(0, bcols, GC):
                            cce = min(cc0 + GC, bcols)
                            ncc = cce - cc0
                            i0 = (b * bcols + cc0) * 8
                            og = hg[:, (b * bcols + cc0) * H:
                                    (b * bcols + cce) * H]
                            nc.gpsimd.dma_gather(
                                out_ap=og.rearrange("p (a b) -> p a b", b=H),
                                in_ap=table[bs:be, :],
                                idxs_ap=it[:, i0:i0 + ncc * 8],
                                num_idxs=ncc * P,
                                num_idxs_reg=ncc * P,
                                elem_size=H)

                    msg = sb.tile([P, SUPER * K * H], BF16, tag="msg")

                    agg = ps_a.tile([P, SUPER * P], F32, tag="agg")
                    # bank-wide zeroing matmul: sets every has_written bit so
                    # all following matmuls accumulate, order-independent
                    nc.tensor.matmul(agg[0:Cin, :nt * P],
                                     lhsT=zrow_s[0:1, 0:Cin],
                                     rhs=zrow_s[0:1, 0:nt * P],
                                     start=True, stop=False)

                    for g0 in range(0, ncols, EGRP):
                        ge = min(g0 + EGRP, ncols)
                        gsz = ge - g0
                        ep = ps_e.tile([P, EGRP * H], F32, tag="ep")
                        # PSUM accumulation groups are per 2KB bank and
                        # start=True zeroes the whole bank: first matmul
                        # into each bank starts it, the id-add closes it.
                        cpb = 512 // Cin   # columns per PSUM bank
                        for j in range(g0, ge):
                            jl = j - g0
                            nc.tensor.matmul(
                                ep[:, jl * Cin:(jl + 1) * Cin],
                                lhsT=ett[:, j * P:(j + 1) * P],
                                rhs=ew_s[l][0:18, :],
                                start=(jl % cpb == 0), stop=False)
                        # accumulate gathered h into the e-PSUM (identity
                        # matmul); split so no matmul output crosses a
                        # 2KB PSUM bank.  hg rows are H-wide even in layer 0
                        # (padded x table), so l==0 adds per column.
                        if l == 0:
                            for j in range(g0, ge):
                                jl = j - g0
                                nc.tensor.matmul(
                                    ep[:, jl * Cin:(jl + 1) * Cin],
                                    lhsT=idb_s[:],
                                    rhs=hg[:, j * H:j * H + Cin],
                                    start=False, stop=(j == ge - 1))
                        else:
                            bank_cols = 512  # f32 elems per partition bank
                            tot = gsz * Cin
                            off = 0
                            while off < tot:
                                seg = min(bank_cols, tot - off)
                                nc.tensor.matmul(
                                    ep[:, off:off + seg],
                                    lhsT=idb_s[:],
                                    rhs=hg[:, g0 * Cin + off:
                                            g0 * Cin + off + seg],
                                    start=False, stop=True)
                                off += seg
                        # fused relu + f32->bf16 PSUM->SBUF
                        nc.scalar.activation(
                            msg[:, g0 * Cin:ge * Cin],
                            ep[:, :gsz * Cin],
                            mybir.ActivationFunctionType.Relu)
                        # one-hot per column (tensor_scalar hits 2x DVE mode)
                        oh = sb3.tile([P, EGRP * P], BF16, tag="oh", name="oh")
                        for j in range(g0, ge):
                            jl = j - g0
                            nc.vector.tensor_scalar(
                                oh[:, jl * P:(jl + 1) * P],
                                iota_s[:, 0:P],
                                dlt[:, j:j + 1],
                                None,
                                op0=mybir.AluOpType.is_equal)
                        for j in range(g0, ge):
                            jl = j - g0
                            t_loc = (j % (nt * KB)) // KB
                            nc.tensor.matmul(
                                agg[0:Cin, t_loc * P:(t_loc + 1) * P],
                                lhsT=msg[:, j * Cin:(j + 1) * Cin],
                                rhs=oh[:, jl * P:(jl + 1) * P],
                                start=False, stop=False)

                    # z = h + agg  (transposing identity matmul)
                    hsrc = xsh_t if l == 0 else hsh[l - 1]
                    hrow = t0 * P
                    hbt = sb.tile([P, SUPER * H], BF16, tag="hb", name="hbt")
                    nc.sync.dma_start(
                        out=hbt[:, :nt * Cin].rearrange(
                            "p (a c) -> p a c", c=Cin),
                        in_=hsrc[hrow:hrow + nt * P, :].rearrange(
                            "(a p) c -> p a c", p=P))
                    for ti in range(nt):
                        nc.tensor.matmul(
                            agg[0:Cin, ti * P:(ti + 1) * P],
                            lhsT=hbt[:, ti * Cin:(ti + 1) * Cin], rhs=idb_s[:],
                            start=False, stop=(ti == nt - 1))

                    # transposed MLP with stationary weights
                    w = nt * P
                    zt = sb.tile([P, SUPER * P], BF16, tag="zt")
                    nc.scalar.copy(zt[0:Cin, :w], agg[0:Cin, :w])
                    y1p = ps_m.tile([P, SUPER * P], F32, tag="mlp")
                    nc.tensor.matmul(y1p[:, :w], lhsT=w1_s[wi][:],
                                     rhs=zt[0:Cin, :w], start=True, stop=True)
                    y1 = sb.tile([P, SUPER * P], BF16, tag="y1")
                    nc.scalar.activation(y1[:, :w], y1p[:, :w],
                                         mybir.ActivationFunctionType.Relu,
                                         bias=ti_s[wi][:], scale=si_s[wi][:])
                    y2p = ps_m.tile([P, SUPER * P], F32, tag="mlp")
                    nc.tensor.matmul(y2p[:, :w], lhsT=w2_s[wi][:],
                                     rhs=y1[:, :w], start=True, stop=True)
                    rt = sb.tile([P, SUPER * P], F32, tag="rt")
                    nc.scalar.activation(rt[:, :w], y2p[:, :w],
                                         mybir.ActivationFunctionType.Relu,
                                         bias=to_s[l][:], scale=so_s[l][:])
                    hpt = sb.tile([P, SUPER * H], BF16, tag="hp", name="hpt")
                    for ti in range(nt):
                        tp = ps_m.tile([P, P], F32, tag="mlp")
                        nc.tensor.transpose(tp[:], rt[:, ti * P:(ti + 1) * P],
                                            idf_s[:])
                        if l == 0:
                            nc.vector.tensor_copy(
                                out=hpt[:, ti * H:(ti + 1) * H], in_=tp[:])
                        else:
                            nc.vector.tensor_add(
                                out=hpt[:, ti * H:(ti + 1) * H], in0=tp[:],
                                in1=hbt[:, ti * Cin:(ti + 1) * Cin])
                    if l < 2:
                        hdst, drow = hsh[l], t0 * P
                    elif t0 * P < RSPLIT:
                        hdst, drow = ptabA, t0 * P
                    else:
                        hdst, drow = ptabB, t0 * P - RSPLIT
                    with tc.high_priority():
                        nc.sync.dma_start(
                            out=hdst[drow:drow + nt * P, :].rearrange(
                                "(a p) c -> p a c", p=P),
                            in_=hpt[:, :nt * H].rearrange(
                                "p (a c) -> p a c", c=H))
                    if l == 2 and t0 * P < W:
                        # duplicate boundary-window rows for the small
                        # AllGather (separate tensor so the collective's
                        # wait covers only these supers)
                        with tc.high_priority():
                            nc.sync.dma_start(
                                out=bnd_in[t0 * P:t0 * P + nt * P, :]
                                .rearrange("(a p) c -> p a c", p=P),
                                in_=hpt[:, :nt * H].rearrange(
                                    "p (a c) -> p a c", c=H))
                    if l < 2 and s == n_super - 1 and R > 1:
                        # one AllGather per layer: a collective blocks the
                        # Pool queue for its whole transfer, so fewer/larger
                        # collectives lose less time.
                        with tc.high_priority():
                            nc.gpsimd.collective_compute(
                                "AllGather", mybir.AluOpType.bypass,
                                replica_groups=RG,
                                ins=[hsh[l][0:NSH, :]],
                                outs=[hfull[l][:, :]])
                if R == 1:
                    if l < 2:
                        nc.sync.dma_start(out=hfull[l][:, :],
                                          in_=hsh[l][0:NSH, :])
                    else:
                        nc.sync.dma_start(out=bnd_full[0:W, :],
                                          in_=bnd_in[0:W, :])

        # ---- mean pool ----
        with ExitStack() as pctx:
            pb = pctx.enter_context(tc.tile_pool(name="poolw", bufs=2))
            pps = pctx.enter_context(tc.tile_pool(name="poolp", bufs=NG,
                                                  space="PSUM"))
            PCH = 8
            PTA, PTB = cfg.PTA, cfg.PTB
            gp = [pps.tile([P, H + 1], F32, tag=f"gp{g}", name=f"gp{g}")
                  for g in range(NG)]
            # chunks never cross a segment boundary; segment A only depends
            # on the first half of layer-2's stores so its pooling overlaps
            # the layer-2 tail.
            chunks = []
            for seg0, segn, src in ((0, PTA, 0), (PTA, PTB, 1),
                                    (PTA + PTB, PTR, 2)):
                for cc in range(0, segn, PCH):
                    chunks.append((src, seg0 + cc, seg0 + min(cc + PCH, segn)))
            total_mm = sum(ce - cs for _, cs, ce in chunks) * NG
            n_mm = 0
            bnd_emitted = False
            for srci, cs, ce in chunks:
                if srci == 2 and not bnd_emitted:
                    # boundary AllGather emitted here so its Pool-queue
                    # stall overlaps the local pool chunks' compute; its
                    # input has been ready since early layer 2.
                    bnd_emitted = True
                    if R > 1:
                        nc.gpsimd.collective_compute(
                            "AllGather", mybir.AluOpType.bypass,
                            replica_groups=RG,
                            ins=[bnd_in[0:W, :]],
                            outs=[bnd_full[:, :]])
                ntl = ce - cs
                pit = pb.tile([P, PCH * 8], I16, tag="pit", name="pit")
                nc.sync.dma_start(out=pit[:, :ntl * 8],
                                  in_=pidx_t[:, cs * 8:ce * 8])
                pg = pb.tile([P, PCH * H], BF16, tag="pg", name="pg")
                ptable = (ptabA, ptabB, bnd_full)[srci]
                nc.gpsimd.dma_gather(
                    out_ap=pg[:, :ntl * H].rearrange("p (a b) -> p a b", b=H),
                    in_ap=ptable[:, :],
                    idxs_ap=pit[:, :ntl * 8],
                    num_idxs=ntl * P,
                    num_idxs_reg=ntl * P,
                    elem_size=H)
                blt = pb.tile([P, PCH * NG], F32, tag="blt", name="blt")
                nc.sync.dma_start(out=blt[:, :ntl * NG],
                                  in_=pbl_t[:, cs * NG:ce * NG])
                for pt in range(cs, ce):
                    off = (pt - cs) * H
                    rhs = pb.tile([P, H + 1], BF16, tag="prhs", name="prhs")
                    nc.vector.tensor_copy(out=rhs[:, 0:H],
                                          in_=pg[:, off:off + H])
                    nc.vector.memset(rhs[:, H:H + 1], 1.0)
                    for g in range(NG):
                        oh = pb.tile([P, P], BF16, tag="poh", name="poh")
                        nc.vector.tensor_scalar(
                            oh[:],
                            iota_s[:, 0:P],
                            blt[:, (pt - cs) * NG + g:(pt - cs) * NG + g + 1],
                            None,
                            op0=mybir.AluOpType.is_equal)
                        n_mm += 1
                        nc.tensor.matmul(
                            gp[g][:, :], lhsT=oh[:], rhs=rhs[:, :],
                            start=(n_mm <= NG), stop=(n_mm > total_mm - NG))
            for g in range(NG):
                rows = min(P, GSH - g * P)
                cnt = pb.tile([P, 1], F32, tag="cnt", name="cnt")
                nc.vector.tensor_scalar_max(cnt[:], gp[g][:, H:H + 1], 1.0)
                rc = pb.tile([P, 1], F32, tag="rc", name="rc")
                nc.vector.reciprocal(rc[:], cnt[:])
                po = pb.tile([P, H], F32, tag="po", name="po")
                nc.vector.tensor_scalar_mul(po[:], gp[g][:, 0:H], rc[:])
                nc.sync.dma_start(out=out_t[g * P:g * P + rows, :],
                                  in_=po[0:rows, :])

    nc.compile()
    return nc


# --------------------------------------------------------------------------
# entry point
# --------------------------------------------------------------------------
LAST_PROFILE = {}


def run(cfg, weights, x_perm, per_core, core_ids=None, trace=False):
    nc = build_program(cfg, weights)
    R = cfg.R
    in_maps = []
    for r in range(R):
        pc = per_core[r]
        in_maps.append({
            "x_perm": x_perm,
            "xsh": pc["xsh"],
            "src_idx": pc["src_idx"],
            "dstloc": pc["dstloc"],
            "eta": pc["eta"],
            "pool_idx": pc["pool_idx"],
            "pool_bl": pc["pool_bl"],
        })
    res = run_bass_kernel_spmd(nc, in_maps,
                               core_ids if core_ids else list(range(R)),
                               trace=trace)
    LAST_PROFILE["exec_time_ns"] = getattr(res, "exec_time_ns", None)
    LAST_PROFILE["res"] = res
    outs = [np.asarray(res.results[r]["out"]) for r in range(R)]
    return np.concatenate(outs, axis=0).astype(np.float32)


def kernel(x, edge_attr,
           mlp0_w1, mlp0_b1, mlp0_bn_g, mlp0_bn_b, mlp0_bn_m, mlp0_bn_v,
           mlp0_w2, mlp0_b2, edge0_w, edge0_b,
           mlps_w1, mlps_b1, mlps_bn_g, mlps_bn_b, mlps_bn_m, mlps_bn_v,
           mlps_w2, mlps_b2, edge_w, edge_b, bn_g, bn_b, bn_m, bn_v,
           edge_index, batch, n_graphs):
    R = 8
    x = np.asarray(x, dtype=np.float32)
    edge_attr = np.asarray(edge_attr, dtype=np.float32)
    N, D0 = x.shape
    H = int(np.asarray(mlps_w1).shape[0])
    G = int(n_graphs)

    cfg, x_perm, per_core = preprocess(x, edge_attr, edge_index, batch, G, R)
    cfg.D0, cfg.H = D0, H

    wts = fold_weights(H, D0,
                       mlp0_w1, mlp0_b1, mlp0_bn_g, mlp0_bn_b, mlp0_bn_m,
                       mlp0_bn_v, mlp0_w2, mlp0_b2, edge0_w, edge0_b,
                       mlps_w1, mlps_b1, mlps_bn_g, mlps_bn_b, mlps_bn_m,
                       mlps_bn_v, mlps_w2, mlps_b2, edge_w, edge_b,
                       bn_g, bn_b, bn_m, bn_v)

    trace = bool(os.environ.get("KBENCH_TRACE"))
    return run(cfg, wts, x_perm, per_core, trace=trace)

